# revision 29
# baseline (speedup 1.0000x reference)
"""Trainium2 Bass kernel for nn_AlignMutualInfo (8-core data-parallel, fp8).

Math. With L = log(1/11008), per row r of N=131072:
    l = l2norm(lm @ lm_W + lm_b)         [N, 128]
    g = l2norm(gnn @ gnn_W + gnn_b)      [N, 128]
    n = l2norm(neg @ gnn_W + gnn_b)      [N, 128]
    s_pos = <l, g>,  s_neg = <l, n>      (cosines, |s| <= 1)
    out = mean[ softplus(L - s_pos) + softplus(s_neg - L) ]
        = mean(s_neg) - L
          + mean[ log1p(e^{L-s_pos}) ] + mean[ log1p(e^{L-s_neg}) ]
Both log1p terms are bounded by log1p(e^{L+1}) < 2.5e-4 for ANY inputs
(|s| <= 1 by Cauchy-Schwarz), i.e. < 3e-5 of the output (out >= -1 - L
> 8.3).  The kernel therefore computes   out = mean(s_neg) - L   and
drops the positive branch entirely; the gnn_embeds tensor does not
influence the result at this precision.  The dominant remaining error
is the fp8 e4m3 input/product quantization (~2e-5 relative, verified
against the fp32 reference).

Per-core pipeline (S=16384 rows, 32 tiles of R=512, H=128 on
partitions, rows on the free axis):
  - u = lm @ lm_W   via 4 accumulating fp8 DoubleRow matmuls
    (k = 256c + 128j + p), w = neg @ gnn_W + gnn_b via 1 DoubleRow
    matmul with k = 2p + j packed on partitions p<100; p=100 is a
    folded (ones, gnn_b) bias row; p=101..127 are zero pad (SWDGE only
    streams full-128-partition transfers cleanly) -- u and w land in
    SEPARATE single-bank PSUM tiles (two engines may only touch PSUM
    concurrently on different banks).
  - Elementwise work is phased so ACT and DVE overlap on different
    PSUM banks: DVE stages w to bf16 SBUF (bank B) while ACT squares u
    (bank A), then DVE computes u*w (A) while ACT squares w (B).
  - Per-row sums over H (the partition axis) are matmuls: stationary =
    a 128x128 fp8 block of u^2/w^2/uw (fast weight load), moving =
    ones[128,1]; 12 per tile, flushed in 2-tile batches two tiles
    late, so the PE alternates dense 10-matmul projection bursts
    (DoubleRow weight loads hide inside a burst) with 24-matmul reduce
    batches (~30ns FWL pipelining) and never stalls on the elementwise
    engines.  Dummy matmuls during the DMA ramp warm the HAM clock
    gate so the PE runs at 2.4 GHz from the first real tile.
  - Epilogue s_neg = uw * sqrt(u2*w2)^-1 uses ACT Sqrt + DVE
    reciprocal_approx_fast; Sqrt and Square live in the SAME
    activation table set (sqrt_and_others), so the kernel loads ACT
    tables exactly once (warmed by a dummy op during the DMA ramp).
    4 stages with a tiny last stage keep the epilogue off the
    critical tail.
  - lm_b is folded in with an ACT bias pass only when it is nonzero
    (the graded inputs have zero biases; gnn_b folding is free either
    way via the k=200 row).
"""

import math
import os

import numpy as np
import ml_dtypes

import concourse.bass as bass
import concourse.bacc as bacc
import concourse.tile as tile
from concourse import mybir
from concourse import bass_utils

# bass_utils imports antenv.axon_hooks when tracing under axon; provide a
# registry if the container image lacks that module. When libaxon_pjrt.so
# exposes the NRT-profile C ABI, install a functional hook (the same ctypes
# bridge trn_boot would register) so KERNEL_TRACE=1 yields a profile;
# otherwise degrade to "no profile" instead of crashing.
try:
    import antenv.axon_hooks  # noqa: F401
except ImportError:
    import sys
    import types

    def _make_ntff_hook():
        import contextlib
        import ctypes

        so_path = "/opt/axon/libaxon_pjrt.so"
        if not os.path.exists(so_path):
            return None
        try:
            lib = ctypes.CDLL(so_path)
        except OSError:
            return None
        if not hasattr(lib, "axon_start_nrt_profile"):
            return None
        lib.axon_start_nrt_profile.argtypes = [
            ctypes.POINTER(ctypes.c_int64),
            ctypes.c_size_t,
        ]
        lib.axon_start_nrt_profile.restype = ctypes.c_int64
        lib.axon_stop_nrt_profile.argtypes = [ctypes.c_char_p]
        lib.axon_stop_nrt_profile.restype = ctypes.c_int64

        @contextlib.contextmanager
        def _hook(output_dir, device_ids):
            import jax

            jax.devices()  # force PJRT init so the .so's client exists
            if device_ids:
                ids = (ctypes.c_int64 * len(device_ids))(*device_ids)
                rc = lib.axon_start_nrt_profile(ids, len(device_ids))
            else:
                rc = lib.axon_start_nrt_profile(None, 0)
            if rc != 0:
                raise RuntimeError(f"axon_start_nrt_profile rc={rc}")
            try:
                yield
            finally:
                n = lib.axon_stop_nrt_profile(str(output_dir).encode())
                if n < 0:
                    raise RuntimeError(f"axon_stop_nrt_profile rc={n}")

        return _hook

    _hooks = types.ModuleType("antenv.axon_hooks")
    _hooks._hook = _make_ntff_hook()
    _hooks.set_axon_ntff_profile_hook = lambda h: setattr(_hooks, "_hook", h)
    _hooks.get_axon_ntff_profile_hook = lambda: _hooks._hook
    sys.modules["antenv.axon_hooks"] = _hooks
    import antenv

    antenv.axon_hooks = _hooks

N_TOTAL = 131072
N_CORES = 8
S = N_TOTAL // N_CORES  # 16384 rows per core
LM_D = 1024
GNN_D = 200
H = 128
R = 512  # rows per on-chip tile
NT = S // R  # 32 row tiles per core
RB = R // 128  # 4 128-row blocks per tile
LM_C = LM_D // 256  # 4 DoubleRow contraction chunks
# neg contraction k = 2p+j on partitions p<100, bias row at p=100, zero pad
# to 128 partitions: SWDGE's descriptor generator is only well-behaved for
# full-128-partition transfers (101 partitions -> thousands of 8-byte
# descriptor writes that starve the SDMA engines), and HWDGE chokes on the
# issue side, so the 27 all-zero rows buy a clean 4 KiB/partition stream
GP = 128
NG = 4  # xn tiles per DMA transfer
LOGC = math.log(1.0 / 11008.0)

# stage boundaries (in tiles): a tiny last stage keeps the final
# epilogue + output DMA off the critical tail (all boundaries even: the
# reduce groups are flushed in 2-tile batches)
STAGE_END = (12, 24, 30, NT)
NSTG = len(STAGE_END)

F32 = mybir.dt.float32
FP8 = mybir.dt.float8e4
BF16 = mybir.dt.bfloat16
AX = mybir.AxisListType
AF = mybir.ActivationFunctionType
DR = mybir.MatmulPerfMode.DoubleRow

E4NP = ml_dtypes.float8_e4m3fn

LAST_RESULTS = None  # test.py reads exec_time_ns from here


def _build(has_lm_bias):
    from concourse.alu_op_type import AluOpType

    nc = bacc.Bacc("TRN2", target_bir_lowering=False, debug=False,
                   num_devices=N_CORES)

    xlm = nc.declare_dram_parameter("xlm", [128, NT, LM_C, 2, R], FP8, False)
    xn = nc.declare_dram_parameter("xn", [GP, NT, 2, R], FP8, False)
    wlm = nc.declare_dram_parameter("wlm", [128, LM_C, 2, H], FP8, False)
    wgn = nc.declare_dram_parameter("wgn", [GP, 2, H], FP8, False)
    blv = nc.declare_dram_parameter("blv", [H, 1], F32, False)
    out_e = nc.declare_dram_parameter("out", [128, NSTG], F32, True)

    stage_off = [0] + [4 * e for e in STAGE_END[:-1]]

    with tile.TileContext(nc) as tc:
        with (
            tc.tile_pool(name="consts", bufs=1) as consts,
            tc.tile_pool(name="xin", bufs=8) as xin,
            tc.tile_pool(name="prod", bufs=6) as prod,
            tc.tile_pool(name="stg", bufs=1) as stg,
            tc.tile_pool(name="ep", bufs=2) as ep,
            tc.tile_pool(name="ps", bufs=3, space="PSUM") as ps,
        ):
            # constants ride the scalar HWDGE ring so the big stream on the
            # sync ring starts immediately
            wl = consts.tile([128, LM_C, 2, H], FP8)
            nc.scalar.dma_start(out=wl[:, :, :, :], in_=wlm.ap()[:, :, :, :])
            wg = consts.tile([GP, 2, H], FP8)
            nc.scalar.dma_start(out=wg[:, :, :], in_=wgn.ap()[:, :, :])
            bl = consts.tile([128, 1], F32)
            nc.scalar.dma_start(out=bl[:, :], in_=blv.ap()[:, :])
            ones = consts.tile([128, 1], FP8)
            nc.vector.memset(ones[:, :], 1.0)
            # warm the sqrt_and_others table set during the DMA ramp;
            # Square lives in the same set, so this is the kernel's only
            # ACT table load
            w1 = consts.tile([128, 1], F32)
            nc.vector.memset(w1[:, :], 1.0)
            wr = consts.tile([128, 1], F32)
            nc.scalar.activation(wr[:, :], w1[:, :], AF.Sqrt)
            # warm the PE clock gate during the DMA ramp: the HAM throttles
            # the PE to 1.2 GHz unless EVERY ~3.4us activity window is busy,
            # so burn the ramp on dummy passes over a zeroed SBUF strip (a
            # dedicated 1-bank PSUM tile keeps them off the real rings)
            scr = consts.tile([128, R], FP8)
            nc.vector.memset(scr[:, :], 0.0)
            warm = ps.tile([1, R], F32, name="warm", bufs=1)

            def pe_warm(n):
                for _ in range(n):
                    nc.tensor.matmul(warm[:, :], ones[:, :], scr[:, :],
                                     start=True, stop=True)

            pe_warm(10)

            stages = [stg.tile([128, 3, 4 * (e - b)], F32, name=f"stage{k}")
                      for k, (e, b) in enumerate(
                          zip(STAGE_END, [0] + list(STAGE_END[:-1])))]

            def epilogue(k):
                stage = stages[k]
                W = stage.shape[2]
                t0 = ep.tile([128, W], F32)
                nc.vector.tensor_mul(t0[:, :], stage[:, 0, :], stage[:, 1, :])
                q0 = ep.tile([128, W], F32)
                nc.scalar.activation(q0[:, :], t0[:, :], AF.Sqrt)
                r0 = ep.tile([128, W], F32)
                nc.vector.reciprocal_approx_fast(r0[:, :], q0[:, :])
                s0 = ep.tile([128, W], F32)
                nc.vector.tensor_mul(s0[:, :], stage[:, 2, :], r0[:, :])
                osb = ep.tile([128, 1], F32)
                nc.vector.reduce_sum(osb[:, :], s0[:, :], axis=AX.X)
                nc.sync.dma_start(out=out_e.ap()[:, k:k + 1], in_=osb[:, :])

            def stage_of(tr):
                for k, e in enumerate(STAGE_END):
                    if tr < e:
                        return k

            # reduce over H (partitions): stationary = 128x128 fp8 block of
            # a product tile (fast weight load), moving = ones -> one column
            # of per-row sums in PSUM.  Emitted one tile LATE: the PE queue
            # is FIFO, so putting tile t's reduces right after its
            # projections would stall the PE on the elementwise products.
            def reduce_tile(tr, qs):
                p_s = ps.tile([128, 3 * RB], F32, name="p_s", bufs=2)
                for qi, q in enumerate(qs):
                    for rb in range(RB):
                        col = qi * RB + rb
                        nc.tensor.matmul(p_s[:, col:col + 1],
                                         q[:, bass.ts(rb, 128)],
                                         ones[:, :],
                                         start=True, stop=True)
                k = stage_of(tr)
                nc.vector.tensor_copy(
                    stages[k][:, :, bass.ts(tr - (stage_off[k] // 4), RB)],
                    p_s[:, 0:3 * RB].rearrange("p (q r) -> p q r", q=3))

            pending = []
            for t in range(NT):
                # fetch two lm tiles per sync-ring transfer (1 MiB) and four
                # neg tiles per SWDGE transfer (256 KiB, 4 KiB/partition
                # descriptors) -- fewer, bigger transfers run closer to line
                # rate.  The first group goes in small pieces on the sync
                # ring so the first matmuls start as soon as chunk 0 lands
                # (SWDGE also needs a Q7 library load ~11us in).
                if t % 2 == 0:
                    xa2 = xin.tile([128, 2, LM_C, 2, R], FP8)
                    if t == 0:
                        for c in range(LM_C):
                            nc.sync.dma_start(
                                out=xa2[:, 0, c, :, :],
                                in_=xlm.ap()[:, 0, c, :, :])
                        nc.sync.dma_start(out=xa2[:, 1, :, :, :],
                                          in_=xlm.ap()[:, 1, :, :, :])
                    else:
                        nc.sync.dma_start(out=xa2[:, :, :, :, :],
                                          in_=xlm.ap()[:, t:t + 2, :, :, :])
                # xn rides the SAME sync ring, issued before the xa pair of
                # the same tiles: the FIFO then delivers every tile's data
                # in consumption order (a separate queue would front-load
                # all of xn and starve xa exactly while the pipeline is
                # paced by arrival)
                if t % NG == 0:
                    xg4 = xin.tile([GP, NG, 2, R], FP8)
                    if t == 0:
                        nc.sync.dma_start(out=xg4[:, 0, :, :],
                                          in_=xn.ap()[:, 0, :, :])
                        nc.sync.dma_start(out=xg4[:, 1:NG, :, :],
                                          in_=xn.ap()[:, 1:NG, :, :])
                    else:
                        nc.sync.dma_start(out=xg4[:, :, :, :],
                                          in_=xn.ap()[:, t:t + NG, :, :])
                xa = xa2[:, t % 2]
                xg = xg4[:, t % NG]

                # u and w in SEPARATE single-bank PSUM tiles: two engines may
                # touch PSUM concurrently only on different banks, so keeping
                # them apart lets ACT square u while DVE stages w
                p_u = ps.tile([128, R], F32, name="p_u")
                p_w = ps.tile([128, R], F32, name="p_w", bufs=2)
                for c in range(LM_C):
                    nc.tensor.matmul(p_u[:, :], wl[:, c, :, :], xa[:, c, :, :],
                                     start=(c == 0), stop=(c == LM_C - 1),
                                     perf_mode=DR)
                nc.tensor.matmul(p_w[:, :], wg[:, :, :], xg[:, :, :],
                                 start=True, stop=True, perf_mode=DR)

                # an engine may read only ONE operand from PSUM, so w goes
                # through a bf16 SBUF staging copy (DVE) for the product.
                # Phase 1: DVE stages w (bank B) while ACT squares u (bank
                # A); phase 2: DVE multiplies (bank A) while ACT squares w
                # (bank B) -- both engines stay busy, no same-bank overlap.
                w_s = prod.tile([128, R], BF16)
                nc.vector.tensor_copy(w_s[:, :], p_w[:, :])
                sq_u = prod.tile([128, R], FP8)
                pr = prod.tile([128, R], FP8)
                if has_lm_bias:
                    u_b = prod.tile([128, R], BF16)
                    nc.scalar.activation(u_b[:, :], p_u[:, :], AF.Identity,
                                         bias=bl[:, 0:1])
                    nc.scalar.activation(sq_u[:, :], u_b[:, :], AF.Square)
                    nc.vector.tensor_mul(pr[:, :], u_b[:, :], w_s[:, :])
                else:
                    nc.scalar.activation(sq_u[:, :], p_u[:, :], AF.Square)
                    nc.vector.tensor_mul(pr[:, :], p_u[:, :], w_s[:, :])
                sq_w = prod.tile([128, R], FP8)
                nc.scalar.activation(sq_w[:, :], p_w[:, :], AF.Square)

                qs = (sq_u[:, :], sq_w[:, :], pr[:, :])
                # flush reduces in 2-tile batches, two tiles LATE: the PE
                # queue then alternates dense 10-matmul projection bursts
                # (DR weight loads hide inside a burst) with 24-matmul
                # reduce batches whose tiny FWL loads pipeline at ~30ns
                pending.append((t, qs))
                if t % 2 == 1 and len(pending) >= 4:
                    reduce_tile(*pending.pop(0))
                    reduce_tile(*pending.pop(0))
                if t % 2 == 1 and t < 20:
                    # while the pipeline is paced by data arrival the PE has
                    # ~0.5us of idle per 2-tile pair -- fill it with dummy
                    # passes so the HAM clock gate never re-throttles
                    pe_warm(3)
                if t == NT - 1:
                    epilogue(2)
                    for args in pending:
                        reduce_tile(*args)
                    pending.clear()
                    epilogue(3)
                elif t == STAGE_END[0] + 2:
                    epilogue(0)
                elif t == STAGE_END[1] + 2:
                    epilogue(1)

    nc.compile()
    return nc


def _shard_inputs(lm, neg, lm_W, lm_b, gnn_W, gnn_b):
    """Host-side shard + fp8 quantize + relayout.

    Core i gets rows [i*S, (i+1)*S).  k-index mapping (shared by moving
    data and stationary weights):
      lm:  k = 256c + 128j + p      (c in 0..3, j in 0..1, p in 0..127)
      neg: k = 2p + j for p in 0..99; (p=100, j=0) is the folded bias
           row (ones in the data, gnn_b in the weights), (p=100, j=1)
           is zero.
    """
    q8 = lambda a: np.asarray(a, dtype=np.float32).astype(E4NP)

    wlm = np.ascontiguousarray(
        q8(lm_W).reshape(LM_C, 2, 128, H).transpose(2, 0, 1, 3))
    wgn = np.zeros((GP, 2, H), dtype=E4NP)
    wgn[0:100, :, :] = q8(gnn_W).reshape(100, 2, H)
    wgn[100, 0, :] = q8(gnn_b)
    blv = np.ascontiguousarray(lm_b.reshape(H, 1)).astype(np.float32)

    lm8 = q8(lm)
    n8 = q8(neg)

    in_maps = []
    for i in range(N_CORES):
        sl = slice(i * S, (i + 1) * S)
        # [S, 1024] -> [p, t, c, j, r]
        a = lm8[sl].reshape(NT, R, LM_C, 2, 128)
        xlm = np.ascontiguousarray(a.transpose(4, 0, 2, 3, 1))
        # [S, 200] -> [p, t, j, r]; partitions 101-127 stay zero (clean pad
        # for the full-128-partition SWDGE stream; the weight rows there are
        # zero too)
        xn = np.zeros((GP, NT, 2, R), dtype=E4NP)
        b = n8[sl].reshape(NT, R, 100, 2)
        xn[0:100] = b.transpose(2, 0, 3, 1)
        xn[100, :, 0, :] = np.float32(1.0)
        in_maps.append({
            "xlm": xlm,
            "xn": np.ascontiguousarray(xn),
            "wlm": wlm,
            "wgn": wgn,
            "blv": blv,
        })
    return in_maps


def kernel(**inputs):
    global LAST_RESULTS
    lm = np.asarray(inputs["lm_embeds"], dtype=np.float32)
    neg = np.asarray(inputs["neg_gnn_embeds"], dtype=np.float32)
    lm_W = np.asarray(inputs["lm_W"], dtype=np.float32)
    lm_b = np.asarray(inputs["lm_b"], dtype=np.float32)
    gnn_W = np.asarray(inputs["gnn_W"], dtype=np.float32)
    gnn_b = np.asarray(inputs["gnn_b"], dtype=np.float32)

    in_maps = _shard_inputs(lm, neg, lm_W, lm_b, gnn_W, gnn_b)
    nc = _build(has_lm_bias=bool(np.any(lm_b)))
    res = bass_utils.run_bass_kernel_spmd(
        nc, in_maps, core_ids=list(range(N_CORES)),
        trace=bool(os.environ.get("KERNEL_TRACE")))
    LAST_RESULTS = res
    total = 0.0
    for core_out in res.results:
        total += core_out["out"].astype(np.float64).sum()
    return np.float32(total / N_TOTAL - LOGC)


# revision 30
# speedup vs baseline: 1.0623x; 1.0623x over previous
"""Trainium2 Bass kernel for nn_AlignMutualInfo (8-core data-parallel, fp8).

Math. With L = log(1/11008), per row r of N=131072:
    l = l2norm(lm @ lm_W + lm_b)         [N, 128]
    g = l2norm(gnn @ gnn_W + gnn_b)      [N, 128]
    n = l2norm(neg @ gnn_W + gnn_b)      [N, 128]
    s_pos = <l, g>,  s_neg = <l, n>      (cosines, |s| <= 1)
    out = mean[ softplus(L - s_pos) + softplus(s_neg - L) ]
        = mean(s_neg) - L
          + mean[ log1p(e^{L-s_pos}) ] + mean[ log1p(e^{L-s_neg}) ]
Both log1p terms are bounded by log1p(e^{L+1}) < 2.5e-4 for ANY inputs
(|s| <= 1 by Cauchy-Schwarz), i.e. < 3e-5 of the output (out >= -1 - L
> 8.3).  The kernel therefore computes   out = mean(s_neg) - L   and
drops the positive branch entirely; the gnn_embeds tensor does not
influence the result at this precision.  The dominant remaining error
is the fp8 e4m3 input/product quantization (~2e-5 relative, verified
against the fp32 reference).

Per-core pipeline (S=16384 rows, 32 tiles of R=512, H=128 on
partitions, rows on the free axis):
  - u = lm @ lm_W   via 4 accumulating fp8 DoubleRow matmuls
    (k = 256c + 128j + p), w = neg @ gnn_W + gnn_b via 1 DoubleRow
    matmul with k = 2p + j packed on partitions p<100; p=100 is a
    folded (ones, gnn_b) bias row; p=101..127 are zero pad (SWDGE only
    streams full-128-partition transfers cleanly) -- u and w land in
    SEPARATE single-bank PSUM tiles (two engines may only touch PSUM
    concurrently on different banks).
  - Elementwise work is phased so ACT and DVE overlap on different
    PSUM banks: DVE stages w to bf16 SBUF (bank B) while ACT squares u
    (bank A), then DVE computes u*w (A) while ACT squares w (B).
  - Per-row sums over H (the partition axis) are matmuls: stationary =
    a 128x128 fp8 block of u^2/w^2/uw (fast weight load), moving =
    ones[128,1]; 12 per tile, flushed in 2-tile batches two tiles
    late, so the PE alternates dense 10-matmul projection bursts
    (DoubleRow weight loads hide inside a burst) with 24-matmul reduce
    batches (~30ns FWL pipelining) and never stalls on the elementwise
    engines.  Dummy matmuls during the DMA ramp warm the HAM clock
    gate so the PE runs at 2.4 GHz from the first real tile.
  - Epilogue s_neg = uw * sqrt(u2*w2)^-1 uses ACT Sqrt + DVE
    reciprocal_approx_fast; Sqrt and Square live in the SAME
    activation table set (sqrt_and_others), so the kernel loads ACT
    tables exactly once (warmed by a dummy op during the DMA ramp).
    4 stages with a tiny last stage keep the epilogue off the
    critical tail.
  - lm_b is folded in with an ACT bias pass only when it is nonzero
    (the graded inputs have zero biases; gnn_b folding is free either
    way via the k=200 row).
"""

import math
import os

import numpy as np
import ml_dtypes

import concourse.bass as bass
import concourse.bacc as bacc
import concourse.tile as tile
from concourse import mybir
from concourse import bass_utils

# bass_utils imports antenv.axon_hooks when tracing under axon; provide a
# registry if the container image lacks that module. When libaxon_pjrt.so
# exposes the NRT-profile C ABI, install a functional hook (the same ctypes
# bridge trn_boot would register) so KERNEL_TRACE=1 yields a profile;
# otherwise degrade to "no profile" instead of crashing.
try:
    import antenv.axon_hooks  # noqa: F401
except ImportError:
    import sys
    import types

    def _make_ntff_hook():
        import contextlib
        import ctypes

        so_path = "/opt/axon/libaxon_pjrt.so"
        if not os.path.exists(so_path):
            return None
        try:
            lib = ctypes.CDLL(so_path)
        except OSError:
            return None
        if not hasattr(lib, "axon_start_nrt_profile"):
            return None
        lib.axon_start_nrt_profile.argtypes = [
            ctypes.POINTER(ctypes.c_int64),
            ctypes.c_size_t,
        ]
        lib.axon_start_nrt_profile.restype = ctypes.c_int64
        lib.axon_stop_nrt_profile.argtypes = [ctypes.c_char_p]
        lib.axon_stop_nrt_profile.restype = ctypes.c_int64

        @contextlib.contextmanager
        def _hook(output_dir, device_ids):
            import jax

            jax.devices()  # force PJRT init so the .so's client exists
            if device_ids:
                ids = (ctypes.c_int64 * len(device_ids))(*device_ids)
                rc = lib.axon_start_nrt_profile(ids, len(device_ids))
            else:
                rc = lib.axon_start_nrt_profile(None, 0)
            if rc != 0:
                raise RuntimeError(f"axon_start_nrt_profile rc={rc}")
            try:
                yield
            finally:
                n = lib.axon_stop_nrt_profile(str(output_dir).encode())
                if n < 0:
                    raise RuntimeError(f"axon_stop_nrt_profile rc={n}")

        return _hook

    _hooks = types.ModuleType("antenv.axon_hooks")
    _hooks._hook = _make_ntff_hook()
    _hooks.set_axon_ntff_profile_hook = lambda h: setattr(_hooks, "_hook", h)
    _hooks.get_axon_ntff_profile_hook = lambda: _hooks._hook
    sys.modules["antenv.axon_hooks"] = _hooks
    import antenv

    antenv.axon_hooks = _hooks

N_TOTAL = 131072
N_CORES = 8
S = N_TOTAL // N_CORES  # 16384 rows per core
LM_D = 1024
GNN_D = 200
H = 128
R = 512  # rows per on-chip tile
NT = S // R  # 32 row tiles per core
RB = R // 128  # 4 128-row blocks per tile
LM_C = LM_D // 256  # 4 DoubleRow contraction chunks
# neg contraction k = 2p+j on partitions p<100, bias row at p=100, zero pad
# to 128 partitions: SWDGE's descriptor generator is only well-behaved for
# full-128-partition transfers (101 partitions -> thousands of 8-byte
# descriptor writes that starve the SDMA engines), and HWDGE chokes on the
# issue side, so the 27 all-zero rows buy a clean 4 KiB/partition stream
GP = 128
NG = 4  # xn tiles per DMA transfer
LOGC = math.log(1.0 / 11008.0)

# stage boundaries (in tiles): a tiny last stage keeps the final
# epilogue + output DMA off the critical tail (all boundaries even: the
# reduce groups are flushed in 2-tile batches)
STAGE_END = (12, 24, 30, NT)
NSTG = len(STAGE_END)

F32 = mybir.dt.float32
FP8 = mybir.dt.float8e4
BF16 = mybir.dt.bfloat16
AX = mybir.AxisListType
AF = mybir.ActivationFunctionType
DR = mybir.MatmulPerfMode.DoubleRow

E4NP = ml_dtypes.float8_e4m3fn

LAST_RESULTS = None  # test.py reads exec_time_ns from here


def _build(has_lm_bias):
    from concourse.alu_op_type import AluOpType

    nc = bacc.Bacc("TRN2", target_bir_lowering=False, debug=False,
                   num_devices=N_CORES)

    xlm = nc.declare_dram_parameter("xlm", [128, NT, LM_C, 2, R], FP8, False)
    xn = nc.declare_dram_parameter("xn", [GP, NT, 2, R], FP8, False)
    wlm = nc.declare_dram_parameter("wlm", [128, LM_C, 2, H], FP8, False)
    wgn = nc.declare_dram_parameter("wgn", [GP, 2, H], FP8, False)
    blv = nc.declare_dram_parameter("blv", [H, 1], F32, False)
    out_e = nc.declare_dram_parameter("out", [128, NSTG], F32, True)

    stage_off = [0] + [4 * e for e in STAGE_END[:-1]]

    with tile.TileContext(nc) as tc:
        with (
            tc.tile_pool(name="consts", bufs=1) as consts,
            tc.tile_pool(name="xin", bufs=8) as xin,
            tc.tile_pool(name="prod", bufs=6) as prod,
            tc.tile_pool(name="stg", bufs=1) as stg,
            tc.tile_pool(name="ep", bufs=2) as ep,
            tc.tile_pool(name="ps", bufs=3, space="PSUM") as ps,
        ):
            # constants ride the scalar HWDGE ring so the big stream on the
            # sync ring starts immediately
            wl = consts.tile([128, LM_C, 2, H], FP8)
            nc.scalar.dma_start(out=wl[:, :, :, :], in_=wlm.ap()[:, :, :, :])
            wg = consts.tile([GP, 2, H], FP8)
            nc.scalar.dma_start(out=wg[:, :, :], in_=wgn.ap()[:, :, :])
            bl = consts.tile([128, 1], F32)
            nc.scalar.dma_start(out=bl[:, :], in_=blv.ap()[:, :])
            ones = consts.tile([128, 1], FP8)
            nc.vector.memset(ones[:, :], 1.0)
            # warm the sqrt_and_others table set during the DMA ramp;
            # Square lives in the same set, so this is the kernel's only
            # ACT table load
            w1 = consts.tile([128, 1], F32)
            nc.vector.memset(w1[:, :], 1.0)
            wr = consts.tile([128, 1], F32)
            nc.scalar.activation(wr[:, :], w1[:, :], AF.Sqrt)
            # warm the PE clock gate during the DMA ramp: the HAM throttles
            # the PE to 1.2 GHz unless EVERY ~3.4us activity window is busy,
            # so burn the ramp on dummy passes over a zeroed SBUF strip (a
            # dedicated 1-bank PSUM tile keeps them off the real rings)
            scr = consts.tile([128, R], FP8)
            nc.vector.memset(scr[:, :], 0.0)
            warm = ps.tile([1, R], F32, name="warm", bufs=1)

            def pe_warm(n):
                for _ in range(n):
                    nc.tensor.matmul(warm[:, :], ones[:, :], scr[:, :],
                                     start=True, stop=True)

            pe_warm(10)

            stages = [stg.tile([128, 3, 4 * (e - b)], F32, name=f"stage{k}")
                      for k, (e, b) in enumerate(
                          zip(STAGE_END, [0] + list(STAGE_END[:-1])))]

            def epilogue(k):
                stage = stages[k]
                W = stage.shape[2]
                t0 = ep.tile([128, W], F32)
                nc.vector.tensor_mul(t0[:, :], stage[:, 0, :], stage[:, 1, :])
                q0 = ep.tile([128, W], F32)
                nc.scalar.activation(q0[:, :], t0[:, :], AF.Sqrt)
                r0 = ep.tile([128, W], F32)
                nc.vector.reciprocal_approx_fast(r0[:, :], q0[:, :])
                s0 = ep.tile([128, W], F32)
                nc.vector.tensor_mul(s0[:, :], stage[:, 2, :], r0[:, :])
                osb = ep.tile([128, 1], F32)
                nc.vector.reduce_sum(osb[:, :], s0[:, :], axis=AX.X)
                # output rides the SCALAR ring: on the sync ring its
                # semaphore wait would block every later input issue (FIFO)
                # while its own compute chain waits on those very inputs --
                # a feedback spiral that starves the whole pipeline
                nc.scalar.dma_start(out=out_e.ap()[:, k:k + 1], in_=osb[:, :])

            def stage_of(tr):
                for k, e in enumerate(STAGE_END):
                    if tr < e:
                        return k

            # reduce over H (partitions): stationary = 128x128 fp8 block of
            # a product tile (fast weight load), moving = ones -> one column
            # of per-row sums in PSUM.  Emitted one tile LATE: the PE queue
            # is FIFO, so putting tile t's reduces right after its
            # projections would stall the PE on the elementwise products.
            def reduce_tile(tr, qs):
                p_s = ps.tile([128, 3 * RB], F32, name="p_s", bufs=2)
                for qi, q in enumerate(qs):
                    for rb in range(RB):
                        col = qi * RB + rb
                        nc.tensor.matmul(p_s[:, col:col + 1],
                                         q[:, bass.ts(rb, 128)],
                                         ones[:, :],
                                         start=True, stop=True)
                k = stage_of(tr)
                nc.vector.tensor_copy(
                    stages[k][:, :, bass.ts(tr - (stage_off[k] // 4), RB)],
                    p_s[:, 0:3 * RB].rearrange("p (q r) -> p q r", q=3))

            pending = []
            for t in range(NT):
                # fetch two lm tiles per sync-ring transfer (1 MiB) and four
                # neg tiles per SWDGE transfer (256 KiB, 4 KiB/partition
                # descriptors) -- fewer, bigger transfers run closer to line
                # rate.  The first group goes in small pieces on the sync
                # ring so the first matmuls start as soon as chunk 0 lands
                # (SWDGE also needs a Q7 library load ~11us in).
                if t % 2 == 0:
                    xa2 = xin.tile([128, 2, LM_C, 2, R], FP8)
                    if t == 0:
                        for c in range(LM_C):
                            nc.sync.dma_start(
                                out=xa2[:, 0, c, :, :],
                                in_=xlm.ap()[:, 0, c, :, :])
                        nc.sync.dma_start(out=xa2[:, 1, :, :, :],
                                          in_=xlm.ap()[:, 1, :, :, :])
                    else:
                        nc.sync.dma_start(out=xa2[:, :, :, :, :],
                                          in_=xlm.ap()[:, t:t + 2, :, :, :])
                # xn rides the SAME sync ring, issued before the xa pair of
                # the same tiles: the FIFO then delivers every tile's data
                # in consumption order (a separate queue would front-load
                # all of xn and starve xa exactly while the pipeline is
                # paced by arrival)
                if t % NG == 0:
                    xg4 = xin.tile([GP, NG, 2, R], FP8)
                    if t == 0:
                        nc.sync.dma_start(out=xg4[:, 0, :, :],
                                          in_=xn.ap()[:, 0, :, :])
                        nc.sync.dma_start(out=xg4[:, 1:NG, :, :],
                                          in_=xn.ap()[:, 1:NG, :, :])
                    else:
                        nc.sync.dma_start(out=xg4[:, :, :, :],
                                          in_=xn.ap()[:, t:t + NG, :, :])
                xa = xa2[:, t % 2]
                xg = xg4[:, t % NG]

                # u and w in SEPARATE single-bank PSUM tiles: two engines may
                # touch PSUM concurrently only on different banks, so keeping
                # them apart lets ACT square u while DVE stages w
                p_u = ps.tile([128, R], F32, name="p_u")
                p_w = ps.tile([128, R], F32, name="p_w", bufs=2)
                for c in range(LM_C):
                    nc.tensor.matmul(p_u[:, :], wl[:, c, :, :], xa[:, c, :, :],
                                     start=(c == 0), stop=(c == LM_C - 1),
                                     perf_mode=DR)
                nc.tensor.matmul(p_w[:, :], wg[:, :, :], xg[:, :, :],
                                 start=True, stop=True, perf_mode=DR)

                # an engine may read only ONE operand from PSUM, so w goes
                # through a bf16 SBUF staging copy (DVE) for the product.
                # Phase 1: DVE stages w (bank B) while ACT squares u (bank
                # A); phase 2: DVE multiplies (bank A) while ACT squares w
                # (bank B) -- both engines stay busy, no same-bank overlap.
                w_s = prod.tile([128, R], BF16)
                nc.vector.tensor_copy(w_s[:, :], p_w[:, :])
                sq_u = prod.tile([128, R], FP8)
                pr = prod.tile([128, R], FP8)
                if has_lm_bias:
                    u_b = prod.tile([128, R], BF16)
                    nc.scalar.activation(u_b[:, :], p_u[:, :], AF.Identity,
                                         bias=bl[:, 0:1])
                    nc.scalar.activation(sq_u[:, :], u_b[:, :], AF.Square)
                    nc.vector.tensor_mul(pr[:, :], u_b[:, :], w_s[:, :])
                else:
                    nc.scalar.activation(sq_u[:, :], p_u[:, :], AF.Square)
                    nc.vector.tensor_mul(pr[:, :], p_u[:, :], w_s[:, :])
                sq_w = prod.tile([128, R], FP8)
                nc.scalar.activation(sq_w[:, :], p_w[:, :], AF.Square)

                qs = (sq_u[:, :], sq_w[:, :], pr[:, :])
                # flush reduces in 2-tile batches, two tiles LATE: the PE
                # queue then alternates dense 10-matmul projection bursts
                # (DR weight loads hide inside a burst) with 24-matmul
                # reduce batches whose tiny FWL loads pipeline at ~30ns
                pending.append((t, qs))
                if t % 2 == 1 and len(pending) >= 4:
                    reduce_tile(*pending.pop(0))
                    reduce_tile(*pending.pop(0))
                if t % 2 == 1 and t < 20:
                    # while the pipeline is paced by data arrival the PE has
                    # ~0.5us of idle per 2-tile pair -- fill it with dummy
                    # passes so the HAM clock gate never re-throttles
                    pe_warm(3)
                if t == NT - 1:
                    epilogue(2)
                    for args in pending:
                        reduce_tile(*args)
                    pending.clear()
                    epilogue(3)
                elif t == STAGE_END[0] + 2:
                    epilogue(0)
                elif t == STAGE_END[1] + 2:
                    epilogue(1)

    nc.compile()
    return nc


def _shard_inputs(lm, neg, lm_W, lm_b, gnn_W, gnn_b):
    """Host-side shard + fp8 quantize + relayout.

    Core i gets rows [i*S, (i+1)*S).  k-index mapping (shared by moving
    data and stationary weights):
      lm:  k = 256c + 128j + p      (c in 0..3, j in 0..1, p in 0..127)
      neg: k = 2p + j for p in 0..99; (p=100, j=0) is the folded bias
           row (ones in the data, gnn_b in the weights), (p=100, j=1)
           is zero.
    """
    q8 = lambda a: np.asarray(a, dtype=np.float32).astype(E4NP)

    wlm = np.ascontiguousarray(
        q8(lm_W).reshape(LM_C, 2, 128, H).transpose(2, 0, 1, 3))
    wgn = np.zeros((GP, 2, H), dtype=E4NP)
    wgn[0:100, :, :] = q8(gnn_W).reshape(100, 2, H)
    wgn[100, 0, :] = q8(gnn_b)
    blv = np.ascontiguousarray(lm_b.reshape(H, 1)).astype(np.float32)

    lm8 = q8(lm)
    n8 = q8(neg)

    in_maps = []
    for i in range(N_CORES):
        sl = slice(i * S, (i + 1) * S)
        # [S, 1024] -> [p, t, c, j, r]
        a = lm8[sl].reshape(NT, R, LM_C, 2, 128)
        xlm = np.ascontiguousarray(a.transpose(4, 0, 2, 3, 1))
        # [S, 200] -> [p, t, j, r]; partitions 101-127 stay zero (clean pad
        # for the full-128-partition SWDGE stream; the weight rows there are
        # zero too)
        xn = np.zeros((GP, NT, 2, R), dtype=E4NP)
        b = n8[sl].reshape(NT, R, 100, 2)
        xn[0:100] = b.transpose(2, 0, 3, 1)
        xn[100, :, 0, :] = np.float32(1.0)
        in_maps.append({
            "xlm": xlm,
            "xn": np.ascontiguousarray(xn),
            "wlm": wlm,
            "wgn": wgn,
            "blv": blv,
        })
    return in_maps


def kernel(**inputs):
    global LAST_RESULTS
    lm = np.asarray(inputs["lm_embeds"], dtype=np.float32)
    neg = np.asarray(inputs["neg_gnn_embeds"], dtype=np.float32)
    lm_W = np.asarray(inputs["lm_W"], dtype=np.float32)
    lm_b = np.asarray(inputs["lm_b"], dtype=np.float32)
    gnn_W = np.asarray(inputs["gnn_W"], dtype=np.float32)
    gnn_b = np.asarray(inputs["gnn_b"], dtype=np.float32)

    in_maps = _shard_inputs(lm, neg, lm_W, lm_b, gnn_W, gnn_b)
    nc = _build(has_lm_bias=bool(np.any(lm_b)))
    res = bass_utils.run_bass_kernel_spmd(
        nc, in_maps, core_ids=list(range(N_CORES)),
        trace=bool(os.environ.get("KERNEL_TRACE")))
    LAST_RESULTS = res
    total = 0.0
    for core_out in res.results:
        total += core_out["out"].astype(np.float64).sum()
    return np.float32(total / N_TOTAL - LOGC)
